# revision 15
# baseline (speedup 1.0000x reference)
"""GPTQ-style grouped-dequant linear on 8 Trainium2 cores.

out[m,n] = sum_k A[m,k] * (q[n,k] - zeros[n,k//128]) * scales[n,k//128] + bias[n]
M=2048, K=4096, N=4096, group=128.

Sharding: column-parallel — qweight/scales/zeros/bias split along N (512/core),
A replicated. Host prep: A cast to bf16 (the kernel computes in bf16 anyway)
and transposed so the contraction dim lands on SBUF partitions; q repacked to
uint8; z/s rows interleaved bf16.

Per core: z/s rows are partition-broadcast by DMA straight from DRAM
(stride-0 source), dequant is two DVE tensor_tensor ops per k-group
producing bf16 W^T tiles in [k,n] layout, then one PSUM-accumulated bf16
matmul chain per 128-row output tile with bias injected as a rank-1 matmul.
DMA issue is spread over the SP/Activation/DVE queues.
"""

import numpy as np

import concourse.bass as bass
import concourse.mybir as mybir
import concourse.tile as tile
from concourse import bacc
from concourse.bass_utils import run_bass_kernel_spmd

P = 128
M, K, N = 2048, 4096, 4096
NCORES = 8
NS = N // NCORES          # 512 out-features per core
G = K // P                # 32 groups (group_size == P == 128)
MT = M // P               # 16 output row tiles

_cached = None


def _build():
    nc = bacc.Bacc("TRN2", target_bir_lowering=False, debug=False,
                   num_devices=NCORES)
    bf16, f32 = mybir.dt.bfloat16, mybir.dt.float32
    at = nc.dram_tensor("AT4", [MT, P, G, P], bf16, kind="ExternalInput")
    qt = nc.dram_tensor("q4", [P, G, NS], mybir.dt.uint8,
                        kind="ExternalInput")
    # z/s rows replicated 16x in DRAM: the partition-broadcast DMA reads 16
    # distinct regions instead of hammering one row (which serializes on a
    # single DRAM page at ~19 B/ns)
    REP = 16
    zs = nc.dram_tensor("zs", [REP, G, 2 * NS], bf16, kind="ExternalInput")
    bi = nc.dram_tensor("bias", [1, NS], bf16, kind="ExternalInput")
    out = nc.dram_tensor("out", [M, NS], f32, kind="ExternalOutput")

    with tile.TileContext(nc) as tc:
        with (
            tc.tile_pool(name="const", bufs=1) as const,
            tc.tile_pool(name="qpool", bufs=1) as qpool,
            tc.tile_pool(name="bcast", bufs=8) as bcp,
            tc.tile_pool(name="bcpsum", bufs=2, space="PSUM") as bcps,
            tc.tile_pool(name="tmp", bufs=3) as tmpp,
            tc.tile_pool(name="wt", bufs=1) as wtp,
            tc.tile_pool(name="apool", bufs=4) as apool,
            tc.tile_pool(name="mpsum", bufs=4, space="PSUM") as mpsum,
            tc.tile_pool(name="opool", bufs=3) as opool,
        ):
            ones = const.tile([1, P], bf16, tag="ones")
            nc.vector.memset(ones, 1.0)
            bias_b = const.tile([1, NS], bf16, tag="bias_b")
            nc.sync.dma_start(out=bias_b[:], in_=bi.ap()[:])

            # The first NB groups' z/s rows are PE-rank-1-broadcast into PSUM
            # (the PE is idle at startup and this keeps 2.5 MB of broadcast
            # traffic off the DMA engines exactly when the lead A tiles and q
            # must land); later groups use DMA partition-broadcast, which has
            # bandwidth to spare once the startup burst is over.
            NB = 10
            # flat z/s rows on partition 0 for the PE broadcasts, 4 chunks
            zsflat = const.tile([1, G, 2, NS], bf16, tag="zsflat")
            zrow = zs.ap()[0:1, :, :]  # [1, G, 2*NS]
            for h in range(4):
                g0, g1 = h * (G // 4), (h + 1) * (G // 4)
                nc.scalar.dma_start(out=zsflat[:, g0:g1, :, :],
                                    in_=zrow[:, g0:g1, :])

            # q^T as [p, g, n]: partition = k%128, one strip per k-group;
            # host layout makes each partition's span fully contiguous.
            q8s = qpool.tile([P, G, NS], mybir.dt.uint8, tag="q8s")
            qr = qt.ap()
            for h in range(8):
                g0, g1 = h * (G // 8), (h + 1) * (G // 8)
                nc.scalar.dma_start(out=q8s[:, g0:g1, :], in_=qr[:, g0:g1, :])

            # DMA partition-broadcasts for the remaining groups
            zs_tiles = {}
            for g in range(NB, G):
                zs_t = bcp.tile([P, 2, NS], bf16)
                src = zs.ap()[:, g, :].unsqueeze(1).to_broadcast(
                    (REP, P // REP, 2 * NS))
                nc.scalar.dma_start(out=zs_t[:], in_=src)
                zs_tiles[g] = zs_t

            atr = at.ap()  # [MT, P, G, P], per-partition contiguous

            def load_ab(mt):
                ab = apool.tile([P, G, P], bf16)
                for h in range(2):
                    g0, g1 = h * (G // 2), (h + 1) * (G // 2)
                    nc.sync.dma_start(out=ab[:, g0:g1, :],
                                      in_=atr[mt, :, g0:g1, :])
                return ab

            def finish(mt, ps):
                ob = opool.tile([P, NS], f32)
                nc.scalar.copy(ob[:], ps[:])
                # out stores ride the idle gpsimd SWDGE queue so they never
                # head-of-line-block the A-tile stream on the SP queue
                nc.gpsimd.dma_start(out=out.ap()[mt * P:(mt + 1) * P, :],
                                    in_=ob[:])

            # Phase 1: dequant each k-group (DMA partition-broadcast of the
            # z/s rows + two DVE ops), consumed by NLEAD concurrently-open
            # PSUM accumulation groups. Leads join progressively as their A
            # strips arrive (catch-up bursts on earlier groups).
            NLEAD = 4
            join_at = {0: 0, 1: 0, 2: 4, 3: 8}
            lead_ab = [load_ab(mt) for mt in range(NLEAD)]
            lead_ps = []
            for mt in range(NLEAD):
                ps = mpsum.tile([P, NS], f32)
                nc.tensor.matmul(ps[:], ones[:], bias_b[:],
                                 start=True, stop=False)
                lead_ps.append(ps)

            zb_ps, sb_ps = {}, {}

            def emit_bcast(g):
                zb = bcps.tile([P, NS], f32)
                sb = bcps.tile([P, NS], f32)
                nc.tensor.matmul(zb[:], ones[:], zsflat[0:1, g, 0, :],
                                 start=True, stop=True)
                nc.tensor.matmul(sb[:], ones[:], zsflat[0:1, g, 1, :],
                                 start=True, stop=True)
                zb_ps[g], sb_ps[g] = zb, sb

            for g in range(min(2, NB)):
                emit_bcast(g)

            wts = []
            for g in range(G):
                # keep the PE bcast matmuls 2 groups ahead of the lead
                # matmuls in the in-order PE queue
                if g + 2 < NB:
                    emit_bcast(g + 2)
                tmp = tmpp.tile([P, NS], bf16)
                wt = wtp.tile([P, NS], bf16, tag=f"wt{g}")
                if g < NB:
                    nc.vector.tensor_tensor(tmp[:], q8s[:, g, :],
                                            zb_ps[g][:],
                                            mybir.AluOpType.subtract)
                    nc.vector.tensor_tensor(wt[:], tmp[:], sb_ps[g][:],
                                            mybir.AluOpType.mult)
                else:
                    zs_t = zs_tiles[g]
                    nc.vector.tensor_tensor(tmp[:], q8s[:, g, :],
                                            zs_t[:, 0, :],
                                            mybir.AluOpType.subtract)
                    nc.vector.tensor_tensor(wt[:], tmp[:], zs_t[:, 1, :],
                                            mybir.AluOpType.mult)
                wts.append(wt)
                for mt in range(NLEAD):
                    if join_at[mt] == g:
                        for gc in range(g + 1):  # catch-up burst
                            nc.tensor.matmul(lead_ps[mt][:],
                                             lead_ab[mt][:, gc, :], wts[gc][:],
                                             start=False,
                                             stop=(gc == G - 1))
                    elif join_at[mt] < g:
                        nc.tensor.matmul(lead_ps[mt][:], lead_ab[mt][:, g, :],
                                         wt[:], start=False,
                                         stop=(g == G - 1))
            for mt in range(NLEAD):
                finish(mt, lead_ps[mt])

            # Phase 2: remaining output tiles, dense back-to-back matmuls
            for mt in range(NLEAD, MT):
                ab = load_ab(mt)
                ps = mpsum.tile([P, NS], f32)
                nc.tensor.matmul(ps[:], ones[:], bias_b[:],
                                 start=True, stop=False)
                for g in range(G):
                    nc.tensor.matmul(ps[:], ab[:, g, :], wts[g][:],
                                     start=False, stop=(g == G - 1))
                finish(mt, ps)

    nc.compile()
    return nc


def _prep_inputs(A, qweight, scales, zeros, bias):
    # AT4[mt, p, g, j] = A[mt*128+j, g*128+p], cast to bf16 (the on-chip
    # pipeline computes the matmul in bf16 regardless)
    bf = mybir.dt.np(mybir.dt.bfloat16)
    at4 = np.ascontiguousarray(
        A.reshape(MT, P, G, P).transpose(0, 3, 2, 1)).astype(bf)
    in_maps = []
    for c in range(NCORES):
        r = slice(c * NS, (c + 1) * NS)
        # q4[p, g, n] = q[n, g*128+p]
        q4 = np.ascontiguousarray(
            qweight[r].astype(np.uint8).T.reshape(G, P, NS).transpose(1, 0, 2))
        zsr = np.empty((G, 2, NS), dtype=np.float32)
        zsr[:, 0, :] = zeros[r].T
        zsr[:, 1, :] = scales[r].T
        zsb = zsr.reshape(G, 2 * NS).astype(bf)
        in_maps.append({
            "AT4": at4,
            "q4": q4,
            "zs": np.ascontiguousarray(
                np.broadcast_to(zsb, (16, G, 2 * NS))),
            "bias": np.ascontiguousarray(bias[r]).reshape(1, NS).astype(bf),
        })
    return in_maps


def run(inputs, **spmd_kwargs):
    global _cached
    if _cached is None:
        _cached = _build()
    in_maps = _prep_inputs(**inputs)
    res = run_bass_kernel_spmd(_cached, in_maps, list(range(NCORES)),
                               **spmd_kwargs)
    outp = np.concatenate([res.results[c]["out"] for c in range(NCORES)],
                          axis=1)
    return outp, res


def kernel(**inputs):
    return run(inputs)[0]


# revision 19
# speedup vs baseline: 1.1713x; 1.1713x over previous
"""GPTQ-style grouped-dequant linear on 8 Trainium2 cores.

out[m,n] = sum_k A[m,k] * (q[n,k] - zeros[n,k//128]) * scales[n,k//128] + bias[n]
M=2048, K=4096, N=4096, group=128.

Sharding: column-parallel — qweight/scales/zeros/bias split along N (512/core),
A replicated. Host prep: A cast to bf16 (the kernel computes in bf16 anyway)
and transposed so the contraction dim lands on SBUF partitions; q repacked to
uint8; z/s rows interleaved bf16.

Per core: z/s rows are partition-broadcast by DMA straight from DRAM
(stride-0 source), dequant is two DVE tensor_tensor ops per k-group
producing bf16 W^T tiles in [k,n] layout, then one PSUM-accumulated bf16
matmul chain per 128-row output tile with bias injected as a rank-1 matmul.
DMA issue is spread over the SP/Activation/DVE queues.
"""

import numpy as np

import concourse.bass as bass
import concourse.mybir as mybir
import concourse.tile as tile
from concourse import bacc
from concourse.bass_utils import run_bass_kernel_spmd

P = 128
M, K, N = 2048, 4096, 4096
NCORES = 8
NS = N // NCORES          # 512 out-features per core
G = K // P                # 32 groups (group_size == P == 128)
MT = M // P               # 16 output row tiles

_cached = None


def _build():
    nc = bacc.Bacc("TRN2", target_bir_lowering=False, debug=False,
                   num_devices=NCORES)
    bf16, f32 = mybir.dt.bfloat16, mybir.dt.float32
    at = nc.dram_tensor("AT4", [MT, P, G, P], bf16, kind="ExternalInput")
    qt = nc.dram_tensor("q4", [P, G, NS], mybir.dt.uint8,
                        kind="ExternalInput")
    # z/s rows replicated 16x in DRAM: the partition-broadcast DMA reads 16
    # distinct regions instead of hammering one row (which serializes on a
    # single DRAM page at ~19 B/ns)
    REP = 16
    zs = nc.dram_tensor("zs", [REP, G, 2 * NS], bf16, kind="ExternalInput")
    bi = nc.dram_tensor("bias", [1, NS], bf16, kind="ExternalInput")
    out = nc.dram_tensor("out", [M, NS], f32, kind="ExternalOutput")

    with tile.TileContext(nc) as tc:
        with (
            tc.tile_pool(name="const", bufs=1) as const,
            tc.tile_pool(name="qpool", bufs=1) as qpool,
            tc.tile_pool(name="bcast", bufs=8) as bcp,
            tc.tile_pool(name="tmp", bufs=3) as tmpp,
            tc.tile_pool(name="wt", bufs=1) as wtp,
            tc.tile_pool(name="apool", bufs=4) as apool,
            tc.tile_pool(name="mpsum", bufs=7, space="PSUM") as mpsum,
            tc.tile_pool(name="opool", bufs=3) as opool,
        ):
            ones = const.tile([1, P], bf16, tag="ones")
            nc.vector.memset(ones, 1.0)
            bias_b = const.tile([1, NS], bf16, tag="bias_b")
            nc.sync.dma_start(out=bias_b[:], in_=bi.ap()[:])

            # q^T as [p, g, n]: partition = k%128, one strip per k-group;
            # host layout makes each partition's span fully contiguous.
            # z/s partition-broadcasts interleave with the q chunks on the
            # Activation queue so early groups' dequant inputs land first.
            q8s = qpool.tile([P, G, NS], mybir.dt.uint8, tag="q8s")
            qr = qt.ap()
            zs_tiles = {}
            for h in range(8):
                g0, g1 = h * (G // 8), (h + 1) * (G // 8)
                nc.scalar.dma_start(out=q8s[:, g0:g1, :], in_=qr[:, g0:g1, :])
                for g in range(g0, g1):
                    zs_t = bcp.tile([P, 2, NS], bf16)
                    src = zs.ap()[:, g, :].unsqueeze(1).to_broadcast(
                        (REP, P // REP, 2 * NS))
                    nc.scalar.dma_start(out=zs_t[:], in_=src)
                    zs_tiles[g] = zs_t

            atr = at.ap()  # [MT, P, G, P], per-partition contiguous

            def load_ab(mt):
                ab = apool.tile([P, G, P], bf16)
                for h in range(2):
                    g0, g1 = h * (G // 2), (h + 1) * (G // 2)
                    nc.sync.dma_start(out=ab[:, g0:g1, :],
                                      in_=atr[mt, :, g0:g1, :])
                return ab

            def finish(mt, ps):
                ob = opool.tile([P, NS], f32)
                nc.scalar.copy(ob[:], ps[:])
                # out stores ride the idle gpsimd SWDGE queue so they never
                # head-of-line-block the A-tile stream on the SP queue
                nc.gpsimd.dma_start(out=out.ap()[mt * P:(mt + 1) * P, :],
                                    in_=ob[:])

            # Phase 1: dequant each k-group (DMA partition-broadcast of the
            # z/s rows + two DVE ops), consumed by NLEAD concurrently-open
            # PSUM accumulation groups. Leads join progressively as their A
            # strips arrive (catch-up bursts on earlier groups).
            NLEAD = 6
            join_at = {0: 0, 1: 0, 2: 2, 3: 4, 4: 6, 5: 8}
            lead_ab = [load_ab(mt) for mt in range(NLEAD)]
            lead_ps = []
            for mt in range(NLEAD):
                ps = mpsum.tile([P, NS], f32)
                nc.tensor.matmul(ps[:], ones[:], bias_b[:],
                                 start=True, stop=False)
                lead_ps.append(ps)

            wts = []
            for g in range(G):
                zs_t = zs_tiles[g]
                tmp = tmpp.tile([P, NS], bf16)
                # subtract on the (otherwise idle) gpsimd engine, multiply
                # on DVE: pipelined wt production at ~1 group/us
                nc.gpsimd.tensor_tensor(tmp[:], q8s[:, g, :], zs_t[:, 0, :],
                                        mybir.AluOpType.subtract)
                wt = wtp.tile([P, NS], bf16, tag=f"wt{g}")
                nc.vector.tensor_tensor(wt[:], tmp[:], zs_t[:, 1, :],
                                        mybir.AluOpType.mult)
                wts.append(wt)
                for mt in range(NLEAD):
                    if join_at[mt] == g:
                        for gc in range(g + 1):  # catch-up burst
                            nc.tensor.matmul(lead_ps[mt][:],
                                             lead_ab[mt][:, gc, :], wts[gc][:],
                                             start=False,
                                             stop=(gc == G - 1))
                    elif join_at[mt] < g:
                        nc.tensor.matmul(lead_ps[mt][:], lead_ab[mt][:, g, :],
                                         wt[:], start=False,
                                         stop=(g == G - 1))
            for mt in range(NLEAD):
                finish(mt, lead_ps[mt])

            # Phase 2: remaining output tiles, dense back-to-back matmuls
            for mt in range(NLEAD, MT):
                ab = load_ab(mt)
                ps = mpsum.tile([P, NS], f32)
                nc.tensor.matmul(ps[:], ones[:], bias_b[:],
                                 start=True, stop=False)
                for g in range(G):
                    nc.tensor.matmul(ps[:], ab[:, g, :], wts[g][:],
                                     start=False, stop=(g == G - 1))
                finish(mt, ps)

    nc.compile()
    return nc


def _prep_inputs(A, qweight, scales, zeros, bias):
    # AT4[mt, p, g, j] = A[mt*128+j, g*128+p], cast to bf16 (the on-chip
    # pipeline computes the matmul in bf16 regardless)
    bf = mybir.dt.np(mybir.dt.bfloat16)
    at4 = np.ascontiguousarray(
        A.reshape(MT, P, G, P).transpose(0, 3, 2, 1)).astype(bf)
    in_maps = []
    for c in range(NCORES):
        r = slice(c * NS, (c + 1) * NS)
        # q4[p, g, n] = q[n, g*128+p]
        q4 = np.ascontiguousarray(
            qweight[r].astype(np.uint8).T.reshape(G, P, NS).transpose(1, 0, 2))
        zsr = np.empty((G, 2, NS), dtype=np.float32)
        zsr[:, 0, :] = zeros[r].T
        zsr[:, 1, :] = scales[r].T
        zsb = zsr.reshape(G, 2 * NS).astype(bf)
        in_maps.append({
            "AT4": at4,
            "q4": q4,
            "zs": np.ascontiguousarray(
                np.broadcast_to(zsb, (16, G, 2 * NS))),
            "bias": np.ascontiguousarray(bias[r]).reshape(1, NS).astype(bf),
        })
    return in_maps


def run(inputs, **spmd_kwargs):
    global _cached
    if _cached is None:
        _cached = _build()
    in_maps = _prep_inputs(**inputs)
    res = run_bass_kernel_spmd(_cached, in_maps, list(range(NCORES)),
                               **spmd_kwargs)
    outp = np.concatenate([res.results[c]["out"] for c in range(NCORES)],
                          axis=1)
    return outp, res


def kernel(**inputs):
    return run(inputs)[0]


# revision 20
# speedup vs baseline: 1.2550x; 1.0715x over previous
"""GPTQ-style grouped-dequant linear on 8 Trainium2 cores.

out[m,n] = sum_k A[m,k] * (q[n,k] - zeros[n,k//128]) * scales[n,k//128] + bias[n]
M=2048, K=4096, N=4096, group=128.

Sharding: column-parallel — qweight/scales/zeros/bias split along N (512/core),
A replicated.

Algebra: out = A @ (q*s)^T - rowsums_g(A) @ (z*s)^T + bias, where
rowsum_g[m] = sum_{k in group g} A[m,k]. The zeros/bias terms collapse into
ONE rank-33 matmul per output tile (lhsT = [rowsums; ones], rhs =
[-(z*s); bias]), so dequant is a single DVE multiply per k-group against a
DMA-partition-broadcast scales tile — no zero-point broadcast at all.

Host prep: A cast to bf16 (the kernel computes in bf16 regardless) and
transposed so the contraction dim lands on SBUF partitions; per-group A row
sums (same single pass over A); q repacked to uint8; small z/s/bias algebra.

Per core: scales rows are partition-broadcast by DMA from 16 DRAM replicas
(a single-row source serializes on one DRAM page), W^T tiles are produced
in [k,n] layout by one DVE multiply each, then one PSUM-accumulated bf16
matmul chain per 128-row output tile, opened by the rank-33 correction
matmul. Staggered lead chains keep the PE fed while W^T tiles stream.
"""

import numpy as np

import concourse.bass as bass
import concourse.mybir as mybir
import concourse.tile as tile
from concourse import bacc
from concourse.bass_utils import run_bass_kernel_spmd

P = 128
M, K, N = 2048, 4096, 4096
NCORES = 8
NS = N // NCORES          # 512 out-features per core
G = K // P                # 32 groups (group_size == P == 128)
MT = M // P               # 16 output row tiles
REP = 16                  # DRAM replicas of the scales rows

_cached = None


def _build():
    nc = bacc.Bacc("TRN2", target_bir_lowering=False, debug=False,
                   num_devices=NCORES)
    bf16, f32 = mybir.dt.bfloat16, mybir.dt.float32
    at = nc.dram_tensor("AT4", [MT, P, G, P], bf16, kind="ExternalInput")
    qt = nc.dram_tensor("q4", [P, G, NS], mybir.dt.uint8,
                        kind="ExternalInput")
    sr = nc.dram_tensor("srep", [REP, G, NS], bf16, kind="ExternalInput")
    rs = nc.dram_tensor("rsum", [G + 1, MT, P], bf16, kind="ExternalInput")
    mz = nc.dram_tensor("mzsb", [G + 1, NS], bf16, kind="ExternalInput")
    out = nc.dram_tensor("out", [M, NS], f32, kind="ExternalOutput")

    with tile.TileContext(nc) as tc:
        with (
            tc.tile_pool(name="const", bufs=1) as const,
            tc.tile_pool(name="qpool", bufs=1) as qpool,
            tc.tile_pool(name="bcast", bufs=8) as bcp,
            tc.tile_pool(name="wt", bufs=1) as wtp,
            tc.tile_pool(name="apool", bufs=5) as apool,
            tc.tile_pool(name="mpsum", bufs=7, space="PSUM") as mpsum,
            tc.tile_pool(name="opool", bufs=3) as opool,
        ):
            # correction operands (tiny, land first)
            rsum_sb = const.tile([G + 1, MT, P], bf16, tag="rsum_sb")
            nc.sync.dma_start(out=rsum_sb[:], in_=rs.ap()[:])
            mzsb = const.tile([G + 1, NS], bf16, tag="mzsb")
            nc.sync.dma_start(out=mzsb[:], in_=mz.ap()[:])

            # q^T as [p, g, n]: partition = k%128, one strip per k-group;
            # host layout makes each partition's span fully contiguous.
            # scales partition-broadcasts interleave with the q chunks on
            # the Activation queue, small chunks first so group 0's inputs
            # land with minimum latency.
            q8s = qpool.tile([P, G, NS], mybir.dt.uint8, tag="q8s")
            qr = qt.ap()
            s_tiles = {}

            def emit_sq(gq):  # one q chunk + the matching s broadcasts
                g0, g1 = gq
                nc.scalar.dma_start(out=q8s[:, g0:g1, :], in_=qr[:, g0:g1, :])
                for g in range(g0, g1):
                    s_t = bcp.tile([P, NS], bf16)
                    src = sr.ap()[:, g, :].unsqueeze(1).to_broadcast(
                        (REP, P // REP, NS))
                    nc.scalar.dma_start(out=s_t[:], in_=src)
                    s_tiles[g] = s_t

            CHUNKS = [(0, 2), (2, 4), (4, 8), (8, 12), (12, 16), (16, 20),
                      (20, 24), (24, 28), (28, 32)]
            for gq in CHUNKS:
                emit_sq(gq)

            atr = at.ap()  # [MT, P, G, P], per-partition contiguous

            def load_ab(mt):
                ab = apool.tile([P, G, P], bf16)
                for h in range(2):
                    g0, g1 = h * (G // 2), (h + 1) * (G // 2)
                    nc.sync.dma_start(out=ab[:, g0:g1, :],
                                      in_=atr[mt, :, g0:g1, :])
                return ab

            def open_chain(mt):
                # rank-33 correction matmul opens the PSUM accumulation:
                # psum = rowsums(A_mt) @ -(z*s) + 1 @ bias
                ps = mpsum.tile([P, NS], f32)
                nc.tensor.matmul(ps[:], rsum_sb[:, mt, :], mzsb[:],
                                 start=True, stop=False)
                return ps

            def finish(mt, ps):
                ob = opool.tile([P, NS], f32)
                nc.scalar.copy(ob[:], ps[:])
                # out stores ride the gpsimd SWDGE queue so they never
                # head-of-line-block the A-tile stream on the SP queue
                nc.gpsimd.dma_start(out=out.ap()[mt * P:(mt + 1) * P, :],
                                    in_=ob[:])

            # Phase 1: per k-group one DVE multiply produces the bf16 W^T
            # tile, consumed by NLEAD concurrently-open PSUM accumulation
            # chains. Leads join progressively (catch-up bursts) as their A
            # strips arrive.
            NLEAD = 6
            join_at = {0: 0, 1: 1, 2: 2, 3: 4, 4: 6, 5: 8}
            lead_ab = [load_ab(mt) for mt in range(NLEAD)]
            lead_ps = [open_chain(mt) for mt in range(NLEAD)]

            wts = []
            for g in range(G):
                wt = wtp.tile([P, NS], bf16, tag=f"wt{g}")
                nc.vector.tensor_tensor(wt[:], q8s[:, g, :], s_tiles[g][:],
                                        mybir.AluOpType.mult)
                wts.append(wt)
                for mt in range(NLEAD):
                    if join_at[mt] == g:
                        for gc in range(g + 1):  # catch-up burst
                            nc.tensor.matmul(lead_ps[mt][:],
                                             lead_ab[mt][:, gc, :], wts[gc][:],
                                             start=False,
                                             stop=(gc == G - 1))
                    elif join_at[mt] < g:
                        nc.tensor.matmul(lead_ps[mt][:], lead_ab[mt][:, g, :],
                                         wt[:], start=False,
                                         stop=(g == G - 1))
            for mt in range(NLEAD):
                finish(mt, lead_ps[mt])

            # Phase 2: remaining output tiles, dense back-to-back matmuls
            for mt in range(NLEAD, MT):
                ab = load_ab(mt)
                ps = open_chain(mt)
                for g in range(G):
                    nc.tensor.matmul(ps[:], ab[:, g, :], wts[g][:],
                                     start=False, stop=(g == G - 1))
                finish(mt, ps)

    nc.compile()
    return nc


def _prep_inputs(A, qweight, scales, zeros, bias):
    bf = mybir.dt.np(mybir.dt.bfloat16)
    # AT4[mt, p, g, j] = A[mt*128+j, g*128+p], cast to bf16 (the on-chip
    # pipeline computes the matmul in bf16 regardless)
    at4 = np.ascontiguousarray(
        A.reshape(MT, P, G, P).transpose(0, 3, 2, 1)).astype(bf)
    # per-group A row sums + ones row: rsum[g, mt, j] = sum_k A_g[mt*128+j]
    rsum = np.empty((G + 1, MT, P), dtype=np.float32)
    rsum[:G] = A.reshape(MT, P, G, P).sum(axis=3).transpose(2, 0, 1)
    rsum[G] = 1.0
    rsum = rsum.astype(bf)
    in_maps = []
    for c in range(NCORES):
        r = slice(c * NS, (c + 1) * NS)
        # q4[p, g, n] = q[n, g*128+p]
        q4 = np.ascontiguousarray(
            qweight[r].astype(np.uint8).T.reshape(G, P, NS).transpose(1, 0, 2))
        sT = scales[r].T.astype(bf)                      # [G, NS]
        mzsb = np.empty((G + 1, NS), dtype=np.float32)
        mzsb[:G] = -(zeros[r] * scales[r]).T             # -(z*s)
        mzsb[G] = bias[r]
        in_maps.append({
            "AT4": at4,
            "q4": q4,
            "srep": np.ascontiguousarray(np.broadcast_to(sT, (REP, G, NS))),
            "rsum": rsum,
            "mzsb": mzsb.astype(bf),
        })
    return in_maps


def run(inputs, **spmd_kwargs):
    global _cached
    if _cached is None:
        _cached = _build()
    in_maps = _prep_inputs(**inputs)
    res = run_bass_kernel_spmd(_cached, in_maps, list(range(NCORES)),
                               **spmd_kwargs)
    outp = np.concatenate([res.results[c]["out"] for c in range(NCORES)],
                          axis=1)
    return outp, res


def kernel(**inputs):
    return run(inputs)[0]
